# revision 26
# baseline (speedup 1.0000x reference)
"""Trainium2 Bass kernel for additive (Bahdanau) attention scores.

Computes scores[b,q,k] = sum_c w_attn[c] * tanh((query@Wq)[b,q,c] + (key@Wk)[b,k,c]) + b_attn
for B=4, Tq=Tk=512, Q=K=1024, C=256, fp32.

Sharding: 8 cores, data-parallel over the 2048 (b,q) rows -> 256 rows/core
(core i handles batch i//2, query rows (i%2)*256..+256). The key projection
for the core's batch is computed on-core (duplicated across the pair of cores
sharing a batch - it is cheap).

Per-core pipeline:
  1. PE matmuls -> q2ctxT (c x q) and k2ctxT (c x k), fp32, c on partitions.
  2. DVE tensor_scalar broadcast-adds (per-partition scalar = q2ctxT column)
     build wide fp32 staging tiles S = q2ctx + k2ctx, 16 slots of (128,512).
  3. ACT tanh over the wide tiles (FD=8192 amortizes the ~224cyc/instr
     overhead; ACT is the bound engine at ~33.5M elems/core), output fp16.
  4. PE matvec per (q, c-chunk): stationary = fp16 sliding-window matrix with
     w_attn chunk in column q, accumulating score rows into a PSUM (128,512).
  5. ACT adds b_attn (per-partition bias) PSUM->SBUF, DMA out.
"""

import sys

if "/opt/trn_rl_repo" not in sys.path:
    sys.path.insert(0, "/opt/trn_rl_repo")

import numpy as np

from concourse import bass, tile, mybir
from concourse.bass_utils import run_bass_kernel_spmd
from concourse.vector_clock import ScopedClock

# Problem shapes (hardcoded per contract).
B, TQ, TK = 4, 512, 512
QDIM, KDIM, C = 1024, 1024, 256
N_CORES = 8
QROWS = (B * TQ) // N_CORES      # 256 query rows per core
NKC = QDIM // 128                # 8 contraction chunks
NCC = C // 128                   # 2 c-chunks
GQ = 16                          # q rows per wide staging tile
NSLOT = GQ * NCC                 # 32 (q, cc) slots per wide tile
WIDE = NSLOT * TK                # 16384 staging free dim

FP32 = mybir.dt.float32
FP16 = mybir.dt.float16


def _patched_drain_and_barrier(self, tick_clock, wait_clock):
    """Split the TileContext tail-drain sem waits across multiple drains.

    The stock exit emits one SP drain carrying a wait per outstanding
    semaphore; walrus codegen on this toolchain rejects >~2 sync waits per
    instruction ("Too many sync wait commands"). One drain per wait encodes
    fine and costs only a few ns at kernel end.
    """
    drain_inst = self.nc.sync.drain()
    wait_clock.add_sem_waits(
        drain_inst.ins, ScopedClock({None: tick_clock.global_clock})
    )
    si = drain_inst.ins.sync_info
    if si is not None and len(si.on_wait) > 1:
        waits = list(si.on_wait)
        upds = list(si.on_update)
        drain_inst.ins.sync_info = mybir.SyncInfo(on_wait=waits[:1], on_update=upds)
        for w in waits[1:]:
            extra = self.nc.sync.drain()
            extra.ins.sync_info = mybir.SyncInfo(on_wait=[w], on_update=[])

    self.nc.all_engine_barrier()
    assert self.sems is not None
    popped = self.nc._tile_sem_poison_stack.pop()
    assert popped is self._sem_poison
    self.nc.clear_and_free_semaphores(list(self.sems.allocated().values()))
    self.nc.all_engine_barrier()


tile.TileContext._drain_and_barrier = _patched_drain_and_barrier

_orig_lower_ordered_insts = tile.TileContext._lower_ordered_insts


def _split_waits_then_lower(self, ordered):
    """Cap sync waits at one per instruction before lowering.

    This walrus build rejects instructions carrying more than ~2 sync waits
    ("Too many sync wait commands"). Hoist all but one wait of each
    instruction onto same-engine NOPs placed immediately before it - the
    engine blocks there instead, which is semantically equivalent (Tile's
    global schedule order guarantees producers precede consumers, so the
    conservative engine-side wait cannot deadlock).
    """
    for bb_name, insts in ordered.items():
        new_insts = []
        changed = False
        for inst in insts:
            si = inst.sync_info
            if si is not None and len(si.on_wait) > 1:
                waits = list(si.on_wait)
                for w in waits[:-1]:
                    nop = mybir.InstNoOp(
                        name=self.nc.get_next_instruction_name(),
                        engine=inst.engine,
                        sync_info=mybir.SyncInfo(on_wait=[w], on_update=[]),
                        bass_nofuse=True,
                    )
                    new_insts.append(nop)
                inst.sync_info = mybir.SyncInfo(
                    on_wait=[waits[-1]], on_update=list(si.on_update)
                )
                changed = True
            new_insts.append(inst)
        if changed:
            insts[:] = new_insts
    return _orig_lower_ordered_insts(self, ordered)


tile.TileContext._lower_ordered_insts = _split_waits_then_lower


def build_program(
    repeat: int = 1,
    loop: int = 1,
    stage_fp16: bool = True,
    in16: bool = True,
    k2_fp16: bool = True,
    gq: int = GQ,
    part: str = "all",
    stage_bufs: int = 2,
    tanh_bufs: int = 2,
    ins_bufs: int = 1,
    ctx_bufs: int = 1,
) -> bass.Bass:
    in_dt = FP16 if in16 else FP32
    nslot = gq * NCC
    wide = nslot * TK

    nc = bass.Bass("TRN2", target_bir_lowering=False, debug=False)

    qT = nc.dram_tensor("qT", [QDIM, QROWS], in_dt, kind="ExternalInput").ap()
    kT = nc.dram_tensor("kT", [KDIM, TK], in_dt, kind="ExternalInput").ap()
    wq = nc.dram_tensor("wq", [QDIM, C], in_dt, kind="ExternalInput").ap()
    wk = nc.dram_tensor("wk", [KDIM, C], in_dt, kind="ExternalInput").ap()
    zw = nc.dram_tensor("zw", [128, NCC * 257], FP16, kind="ExternalInput").ap()
    bb = nc.dram_tensor("bb", [128, 1], FP32, kind="ExternalInput").ap()
    out = nc.dram_tensor("out", [QROWS, TK], FP32, kind="ExternalOutput").ap()

    import contextlib

    with tile.TileContext(nc) as tc:
      with (tc.For_i(0, loop, 1) if loop > 1 else contextlib.nullcontext()):
       with (
            tc.tile_pool(name="ins", bufs=ins_bufs) as ins_pool,
            tc.tile_pool(name="ctx", bufs=ctx_bufs) as ctx_pool,
            tc.tile_pool(name="stage", bufs=stage_bufs) as stage_pool,
            tc.tile_pool(name="tanh", bufs=tanh_bufs) as tanh_pool,
            tc.tile_pool(name="scores", bufs=2) as sc_pool,
            tc.tile_pool(name="psum_proj", bufs=2, space="PSUM") as pp_pool,
            tc.tile_pool(name="psum_sc", bufs=2, space="PSUM") as ps_pool,
       ):
        for _rep in range(repeat):
            if part == "main":
                # timing decomposition: skip loads+projections, memset ctx
                zw_sb = ins_pool.tile([128, NCC * 257], FP16, tag="zw")
                nc.vector.memset(zw_sb[:], 0.25)
                bb_sb = ins_pool.tile([128, 1], FP32, tag="bb")
                nc.vector.memset(bb_sb[:], 0.01)
                q2 = []
                k2 = []
                for cc in range(NCC):
                    t = ctx_pool.tile([128, QROWS], FP32, tag=f"q2{cc}")
                    nc.vector.memset(t[:], 0.5)
                    q2.append(t)
                    t = ctx_pool.tile(
                        [128, TK], FP16 if k2_fp16 else FP32, tag=f"k2{cc}"
                    )
                    nc.vector.memset(t[:], 0.5)
                    k2.append(t)

            # ---- loads ----
            qT_sb = []
            kT_sb = []
            wq_sb = []
            wk_sb = []
            for kc in range(NKC) if part != "main" else []:
                t = ins_pool.tile([128, QROWS], in_dt, tag=f"qT{kc}")
                nc.sync.dma_start(t[:], qT[kc * 128:(kc + 1) * 128, :])
                qT_sb.append(t)
                t = ins_pool.tile([128, TK], in_dt, tag=f"kT{kc}")
                nc.sync.dma_start(t[:], kT[kc * 128:(kc + 1) * 128, :])
                kT_sb.append(t)
                t = ins_pool.tile([128, C], in_dt, tag=f"wq{kc}")
                nc.sync.dma_start(t[:], wq[kc * 128:(kc + 1) * 128, :])
                wq_sb.append(t)
                t = ins_pool.tile([128, C], in_dt, tag=f"wk{kc}")
                nc.sync.dma_start(t[:], wk[kc * 128:(kc + 1) * 128, :])
                wk_sb.append(t)
            if part != "main":
                zw_sb = ins_pool.tile([128, NCC * 257], FP16, tag="zw")
                nc.sync.dma_start(zw_sb[:], zw[:])
                bb_sb = ins_pool.tile([128, 1], FP32, tag="bb")
                nc.sync.dma_start(bb_sb[:], bb[:])
                q2 = []
                k2 = []

            # ---- projections: q2ctxT (c x q), k2ctxT (c x k), c on partitions ----
            for cc in range(NCC) if part != "main" else []:
                pq = pp_pool.tile([128, QROWS], FP32, tag="pq")
                for kc in range(NKC):
                    nc.tensor.matmul(
                        pq[:],
                        wq_sb[kc][:, cc * 128:(cc + 1) * 128],
                        qT_sb[kc][:],
                        start=(kc == 0),
                        stop=(kc == NKC - 1),
                    )
                t = ctx_pool.tile([128, QROWS], FP32, tag=f"q2{cc}")
                nc.vector.tensor_copy(t[:], pq[:])
                q2.append(t)

                pk = pp_pool.tile([128, TK], FP32, tag="pk")
                for kc in range(NKC):
                    nc.tensor.matmul(
                        pk[:],
                        wk_sb[kc][:, cc * 128:(cc + 1) * 128],
                        kT_sb[kc][:],
                        start=(kc == 0),
                        stop=(kc == NKC - 1),
                    )
                t = ctx_pool.tile(
                    [128, TK], FP16 if k2_fp16 else FP32, tag=f"k2{cc}"
                )
                nc.vector.tensor_copy(t[:], pk[:])
                k2.append(t)

            if part == "prologue":
                # consume k2/q2 so nothing is dead
                sc = sc_pool.tile([128, TK], FP32, tag="sc")
                nc.vector.tensor_copy(sc[:], k2[0][:])
                nc.vector.tensor_copy(sc[:, :QROWS], q2[0][:])
                nc.sync.dma_start(out[0:128, :], sc[:])
                continue

            # ---- main loop ----
            no_dve = part in ("act", "act_pe")
            no_pe = part in ("act", "act_dve")
            s_static = None
            n_groups = 128 // gq  # groups per q-block
            for qb in range(QROWS // 128):
                if not no_pe:
                    psum = ps_pool.tile([128, TK], FP32, tag="psc")
                for g in range(n_groups):
                    qbase = qb * 128 + g * gq
                    if no_dve:
                        if s_static is None:
                            s_static = stage_pool.tile(
                                [128, wide], FP16 if stage_fp16 else FP32,
                                tag="stage",
                            )
                            nc.vector.memset(s_static[:], 0.125)
                        s_t = s_static
                    else:
                        s_t = stage_pool.tile(
                            [128, wide], FP16 if stage_fp16 else FP32, tag="stage"
                        )
                        for s in range(nslot):
                            q = qbase + s // NCC
                            cc = s % NCC
                            nc.vector.tensor_scalar_add(
                                s_t[:, s * TK:(s + 1) * TK],
                                k2[cc][:],
                                q2[cc][:, q:q + 1],
                            )
                    t_t = tanh_pool.tile([128, wide], FP16, tag="tanh")
                    nc.scalar.activation(
                        t_t[:], s_t[:], mybir.ActivationFunctionType.Tanh
                    )
                    if no_pe:
                        # keep each tanh tile alive with a tiny probe read
                        probe = sc_pool.tile([128, 1], FP32, tag="probe")
                        nc.vector.tensor_copy(probe[:], t_t[:, :1])
                    else:
                        for s in range(nslot):
                            q = qbase + s // NCC
                            cc = s % NCC
                            qi = q - qb * 128
                            zoff = cc * 257 + 128 - qi
                            nc.tensor.matmul(
                                psum[:],
                                zw_sb[:, zoff:zoff + 128],
                                t_t[:, s * TK:(s + 1) * TK],
                                start=(g == 0 and s == 0),
                                stop=(g == n_groups - 1 and s == nslot - 1),
                            )
                sc = sc_pool.tile([128, TK], FP32, tag="sc")
                if no_pe:
                    nc.vector.tensor_copy(sc[:], t_t[:, :TK])
                else:
                    nc.vector.tensor_scalar_add(sc[:], psum[:], bb_sb[:])
                nc.sync.dma_start(out[qb * 128:(qb + 1) * 128, :], sc[:])
            s_static = None

    return nc


class SpmdRunner:
    """Persistent 8-core runner: jit/load the NEFF once, re-invoke cheaply.

    run_bass_kernel_spmd under axon rebuilds the jax.jit closure every call,
    so every invocation re-ships and re-loads the NEFF. Keeping the jitted
    executable alive makes repeated kernel() calls cost only dispatch +
    transfer + execution.
    """

    def __init__(self, nc: bass.Bass, n_cores: int, chain: int = 1):
        import jax
        from concourse import bass2jax
        from jax.experimental.shard_map import shard_map
        from jax.sharding import Mesh, PartitionSpec

        bass2jax.install_neuronx_cc_hook()
        self.jax = jax
        self.nc = nc
        self.n_cores = n_cores
        self.PartitionSpec = PartitionSpec

        partition_name = (
            nc.partition_id_tensor.name if nc.partition_id_tensor else None
        )
        in_names, out_names, out_avals, zero_outs = [], [], [], []
        for alloc in nc.m.functions[0].allocations:
            if not isinstance(alloc, mybir.MemoryLocationSet):
                continue
            name = alloc.memorylocations[0].name
            if alloc.kind == "ExternalInput":
                if name != partition_name:
                    in_names.append(name)
            elif alloc.kind == "ExternalOutput":
                out_names.append(name)
                shape = tuple(alloc.tensor_shape)
                dtype = mybir.dt.np(alloc.dtype)
                out_avals.append(jax.core.ShapedArray(shape, dtype))
                zero_outs.append(np.zeros(shape, dtype))
        self.in_names = list(in_names)
        self.out_names = out_names
        self.out_avals = out_avals
        self.zero_outs = zero_outs
        n_params = len(in_names)
        n_outs = len(out_avals)
        all_in_names = list(in_names) + list(out_names)
        if partition_name is not None:
            all_in_names.append(partition_name)

        def _exec(operands):
            if partition_name is not None:
                operands = operands + [bass2jax.partition_id_tensor()]
            return bass2jax._bass_exec_p.bind(
                *operands,
                out_avals=tuple(out_avals),
                in_names=tuple(all_in_names),
                out_names=tuple(out_names),
                lowering_input_output_aliases=(),
                sim_require_finite=True,
                sim_require_nnan=True,
                nc=nc,
            )

        def _body(*args):
            ins = list(args[:n_params])
            outs = list(args[n_params:])
            # Chain NEFF executions inside one dispatch: each iteration's
            # outputs seed the next call's output operands, creating a data
            # dependence so XLA cannot CSE or reorder the calls. The kernel
            # overwrites every output element, so results are unchanged.
            for _ in range(chain):
                outs = list(_exec(ins + outs))
            return tuple(outs)

        devices = jax.devices()[:n_cores]
        assert len(devices) == n_cores
        self.mesh = Mesh(np.asarray(devices), ("core",))
        in_specs = (PartitionSpec("core"),) * (n_params + n_outs)
        out_specs = (PartitionSpec("core"),) * n_outs
        self.sharded = jax.jit(
            shard_map(
                _body,
                mesh=self.mesh,
                in_specs=in_specs,
                out_specs=out_specs,
                check_rep=False,
            ),
            keep_unused=True,
        )
        self._zeros_dev = None

    def set_inputs(self, in_maps):
        jax = self.jax
        concat_in = [
            np.concatenate(
                [np.asarray(in_maps[c][name]) for c in range(self.n_cores)], axis=0
            )
            for name in self.in_names
        ]
        sharding = jax.sharding.NamedSharding(self.mesh, self.PartitionSpec("core"))
        dev_in = [jax.device_put(a, sharding) for a in concat_in]
        if self._zeros_dev is None:
            concat_zeros = [
                np.zeros((self.n_cores * z.shape[0], *z.shape[1:]), z.dtype)
                for z in self.zero_outs
            ]
            self._zeros_dev = [jax.device_put(a, sharding) for a in concat_zeros]
        self._dev_args = dev_in + self._zeros_dev
        jax.block_until_ready(self._dev_args)

    def run(self):
        out_arrs = self.sharded(*self._dev_args)
        self.jax.block_until_ready(out_arrs)
        return out_arrs

    def results(self, out_arrs):
        res = []
        for c in range(self.n_cores):
            res.append(
                {
                    name: np.asarray(out_arrs[i]).reshape(
                        self.n_cores, *self.out_avals[i].shape
                    )[c]
                    for i, name in enumerate(self.out_names)
                }
            )
        return res


_RUNNER_CACHE = None


def _get_runner():
    global _RUNNER_CACHE
    if _RUNNER_CACHE is None:
        _RUNNER_CACHE = SpmdRunner(build_program(), N_CORES)
    return _RUNNER_CACHE


def make_in_maps(query, key, Wq, Wk, w_attn, b_attn, in16: bool = True):
    in_np = np.float16 if in16 else np.float32
    w16 = np.asarray(w_attn, dtype=np.float16)
    zw = np.zeros((128, NCC * 257), dtype=np.float16)
    for cc in range(NCC):
        zw[:, cc * 257 + 128] = w16[cc * 128:(cc + 1) * 128]
    bbv = np.full((128, 1), np.float32(b_attn), dtype=np.float32)
    wq = np.ascontiguousarray(np.asarray(Wq, dtype=in_np))
    wk = np.ascontiguousarray(np.asarray(Wk, dtype=in_np))

    in_maps = []
    for i in range(N_CORES):
        b = i // 2
        h = i % 2
        qs = np.ascontiguousarray(
            np.asarray(query[b, h * QROWS:(h + 1) * QROWS, :], dtype=in_np).T
        )
        ks = np.ascontiguousarray(np.asarray(key[b], dtype=in_np).T)
        in_maps.append(
            {"qT": qs, "kT": ks, "wq": wq, "wk": wk, "zw": zw, "bb": bbv}
        )
    return in_maps


def kernel(query, key, Wq, Wk, w_attn, b_attn):
    r = _get_runner()
    in_maps = make_in_maps(query, key, Wq, Wk, w_attn, b_attn)
    r.set_inputs(in_maps)
    res = r.results(r.run())
    scores = np.empty((B, TQ, TK), dtype=np.float32)
    for i in range(N_CORES):
        b = i // 2
        h = i % 2
        scores[b, h * QROWS:(h + 1) * QROWS, :] = res[i]["out"]
    return scores


# revision 28
# speedup vs baseline: 1.1061x; 1.1061x over previous
"""Trainium2 Bass kernel for additive (Bahdanau) attention scores.

Computes scores[b,q,k] = sum_c w_attn[c] * tanh((query@Wq)[b,q,c] + (key@Wk)[b,k,c]) + b_attn
for B=4, Tq=Tk=512, Q=K=1024, C=256, fp32.

Sharding: 8 cores, data-parallel over the 2048 (b,q) rows -> 256 rows/core
(core i handles batch i//2, query rows (i%2)*256..+256). The key projection
for the core's batch is computed on-core (duplicated across the pair of cores
sharing a batch - it is cheap).

Per-core pipeline:
  1. PE matmuls -> q2ctxT (c x q) and k2ctxT (c x k), fp32, c on partitions.
  2. DVE tensor_scalar broadcast-adds (per-partition scalar = q2ctxT column)
     build wide fp32 staging tiles S = q2ctx + k2ctx, 16 slots of (128,512).
  3. ACT tanh over the wide tiles (FD=8192 amortizes the ~224cyc/instr
     overhead; ACT is the bound engine at ~33.5M elems/core), output fp16.
  4. PE matvec per (q, c-chunk): stationary = fp16 sliding-window matrix with
     w_attn chunk in column q, accumulating score rows into a PSUM (128,512).
  5. ACT adds b_attn (per-partition bias) PSUM->SBUF, DMA out.
"""

import sys

if "/opt/trn_rl_repo" not in sys.path:
    sys.path.insert(0, "/opt/trn_rl_repo")

import numpy as np

from concourse import bass, tile, mybir
from concourse.bass_utils import run_bass_kernel_spmd
from concourse.vector_clock import ScopedClock

# Problem shapes (hardcoded per contract).
B, TQ, TK = 4, 512, 512
QDIM, KDIM, C = 1024, 1024, 256
N_CORES = 8
QROWS = (B * TQ) // N_CORES      # 256 query rows per core
NKC = QDIM // 128                # 8 contraction chunks
NCC = C // 128                   # 2 c-chunks
GQ = 16                          # q rows per wide staging tile
NSLOT = GQ * NCC                 # 32 (q, cc) slots per wide tile
WIDE = NSLOT * TK                # 16384 staging free dim

FP32 = mybir.dt.float32
FP16 = mybir.dt.float16


def _patched_drain_and_barrier(self, tick_clock, wait_clock):
    """Split the TileContext tail-drain sem waits across multiple drains.

    The stock exit emits one SP drain carrying a wait per outstanding
    semaphore; walrus codegen on this toolchain rejects >~2 sync waits per
    instruction ("Too many sync wait commands"). One drain per wait encodes
    fine and costs only a few ns at kernel end.
    """
    drain_inst = self.nc.sync.drain()
    wait_clock.add_sem_waits(
        drain_inst.ins, ScopedClock({None: tick_clock.global_clock})
    )
    si = drain_inst.ins.sync_info
    if si is not None and len(si.on_wait) > 1:
        waits = list(si.on_wait)
        upds = list(si.on_update)
        drain_inst.ins.sync_info = mybir.SyncInfo(on_wait=waits[:1], on_update=upds)
        for w in waits[1:]:
            extra = self.nc.sync.drain()
            extra.ins.sync_info = mybir.SyncInfo(on_wait=[w], on_update=[])

    self.nc.all_engine_barrier()
    assert self.sems is not None
    popped = self.nc._tile_sem_poison_stack.pop()
    assert popped is self._sem_poison
    self.nc.clear_and_free_semaphores(list(self.sems.allocated().values()))
    self.nc.all_engine_barrier()


tile.TileContext._drain_and_barrier = _patched_drain_and_barrier

_orig_lower_ordered_insts = tile.TileContext._lower_ordered_insts


def _split_waits_then_lower(self, ordered):
    """Cap sync waits at one per instruction before lowering.

    This walrus build rejects instructions carrying more than ~2 sync waits
    ("Too many sync wait commands"). Hoist all but one wait of each
    instruction onto same-engine NOPs placed immediately before it - the
    engine blocks there instead, which is semantically equivalent (Tile's
    global schedule order guarantees producers precede consumers, so the
    conservative engine-side wait cannot deadlock).
    """
    for bb_name, insts in ordered.items():
        new_insts = []
        changed = False
        for inst in insts:
            si = inst.sync_info
            if si is not None and len(si.on_wait) > 1:
                waits = list(si.on_wait)
                for w in waits[:-1]:
                    nop = mybir.InstNoOp(
                        name=self.nc.get_next_instruction_name(),
                        engine=inst.engine,
                        sync_info=mybir.SyncInfo(on_wait=[w], on_update=[]),
                        bass_nofuse=True,
                    )
                    new_insts.append(nop)
                inst.sync_info = mybir.SyncInfo(
                    on_wait=[waits[-1]], on_update=list(si.on_update)
                )
                changed = True
            new_insts.append(inst)
        if changed:
            insts[:] = new_insts
    return _orig_lower_ordered_insts(self, ordered)


tile.TileContext._lower_ordered_insts = _split_waits_then_lower


def _act_immediate(nc, out_ap, in_ap, func=None):
    """ACTIVATE with immediate bias/scale/alpha operands.

    bass forces a per-partition const-AP bias for non-Copy functions; the AP
    read costs ~260ns/instruction on HW. Walrus accepts immediate operands
    fine (verified numerically on HW), saving ~5-8us per kernel iteration.
    """
    func = func or mybir.ActivationFunctionType.Tanh
    eng = nc.scalar
    ins = [eng.lower_ap(in_ap)]
    for v in (0.0, 1.0, 0.0):  # bias, scale, alpha
        ins.append(mybir.ImmediateValue(dtype=FP32, value=v))
    return eng.add_instruction(
        mybir.InstActivation(
            name=nc.get_next_instruction_name(),
            func=func,
            ins=ins,
            outs=[eng.lower_ap(out_ap)],
        )
    )


def build_program(
    repeat: int = 1,
    loop: int = 1,
    stage_fp16: bool = True,
    in16: bool = True,
    k2_fp16: bool = True,
    gq: int = GQ,
    part: str = "all",
    stage_bufs: int = 2,
    tanh_bufs: int = 2,
    ins_bufs: int = 1,
    ctx_bufs: int = 1,
) -> bass.Bass:
    in_dt = FP16 if in16 else FP32
    nslot = gq * NCC
    wide = nslot * TK

    nc = bass.Bass("TRN2", target_bir_lowering=False, debug=False)

    qT = nc.dram_tensor("qT", [QDIM, QROWS], in_dt, kind="ExternalInput").ap()
    kT = nc.dram_tensor("kT", [KDIM, TK], in_dt, kind="ExternalInput").ap()
    wq = nc.dram_tensor("wq", [QDIM, C], in_dt, kind="ExternalInput").ap()
    wk = nc.dram_tensor("wk", [KDIM, C], in_dt, kind="ExternalInput").ap()
    zw = nc.dram_tensor("zw", [128, NCC * 257], FP16, kind="ExternalInput").ap()
    bb = nc.dram_tensor("bb", [128, 1], FP32, kind="ExternalInput").ap()
    out = nc.dram_tensor("out", [QROWS, TK], FP32, kind="ExternalOutput").ap()

    import contextlib

    with tile.TileContext(nc) as tc:
      with (tc.For_i(0, loop, 1) if loop > 1 else contextlib.nullcontext()):
       with (
            tc.tile_pool(name="ins", bufs=ins_bufs) as ins_pool,
            tc.tile_pool(name="ctx", bufs=ctx_bufs) as ctx_pool,
            tc.tile_pool(name="stage", bufs=stage_bufs) as stage_pool,
            tc.tile_pool(name="tanh", bufs=tanh_bufs) as tanh_pool,
            tc.tile_pool(name="scores", bufs=2) as sc_pool,
            tc.tile_pool(name="psum_proj", bufs=2, space="PSUM") as pp_pool,
            tc.tile_pool(name="psum_sc", bufs=2, space="PSUM") as ps_pool,
       ):
        for _rep in range(repeat):
            if part == "main":
                # timing decomposition: skip loads+projections, memset ctx
                zw_sb = ins_pool.tile([128, NCC * 257], FP16, tag="zw")
                nc.vector.memset(zw_sb[:], 0.25)
                bb_sb = ins_pool.tile([128, 1], FP32, tag="bb")
                nc.vector.memset(bb_sb[:], 0.01)
                q2 = []
                k2 = []
                for cc in range(NCC):
                    t = ctx_pool.tile([128, QROWS], FP32, tag=f"q2{cc}")
                    nc.vector.memset(t[:], 0.5)
                    q2.append(t)
                    t = ctx_pool.tile(
                        [128, TK], FP16 if k2_fp16 else FP32, tag=f"k2{cc}"
                    )
                    nc.vector.memset(t[:], 0.5)
                    k2.append(t)

            # ---- loads ----
            qT_sb = []
            kT_sb = []
            wq_sb = []
            wk_sb = []
            for kc in range(NKC) if part != "main" else []:
                t = ins_pool.tile([128, QROWS], in_dt, tag=f"qT{kc}")
                nc.sync.dma_start(t[:], qT[kc * 128:(kc + 1) * 128, :])
                qT_sb.append(t)
                t = ins_pool.tile([128, TK], in_dt, tag=f"kT{kc}")
                nc.sync.dma_start(t[:], kT[kc * 128:(kc + 1) * 128, :])
                kT_sb.append(t)
                t = ins_pool.tile([128, C], in_dt, tag=f"wq{kc}")
                nc.sync.dma_start(t[:], wq[kc * 128:(kc + 1) * 128, :])
                wq_sb.append(t)
                t = ins_pool.tile([128, C], in_dt, tag=f"wk{kc}")
                nc.sync.dma_start(t[:], wk[kc * 128:(kc + 1) * 128, :])
                wk_sb.append(t)
            if part != "main":
                zw_sb = ins_pool.tile([128, NCC * 257], FP16, tag="zw")
                nc.sync.dma_start(zw_sb[:], zw[:])
                bb_sb = ins_pool.tile([128, 1], FP32, tag="bb")
                nc.sync.dma_start(bb_sb[:], bb[:])
                q2 = []
                k2 = []

            # ---- projections: q2ctxT (c x q), k2ctxT (c x k), c on partitions ----
            for cc in range(NCC) if part != "main" else []:
                pq = pp_pool.tile([128, QROWS], FP32, tag="pq")
                for kc in range(NKC):
                    nc.tensor.matmul(
                        pq[:],
                        wq_sb[kc][:, cc * 128:(cc + 1) * 128],
                        qT_sb[kc][:],
                        start=(kc == 0),
                        stop=(kc == NKC - 1),
                    )
                t = ctx_pool.tile([128, QROWS], FP32, tag=f"q2{cc}")
                nc.vector.tensor_copy(t[:], pq[:])
                q2.append(t)

                pk = pp_pool.tile([128, TK], FP32, tag="pk")
                for kc in range(NKC):
                    nc.tensor.matmul(
                        pk[:],
                        wk_sb[kc][:, cc * 128:(cc + 1) * 128],
                        kT_sb[kc][:],
                        start=(kc == 0),
                        stop=(kc == NKC - 1),
                    )
                t = ctx_pool.tile(
                    [128, TK], FP16 if k2_fp16 else FP32, tag=f"k2{cc}"
                )
                nc.vector.tensor_copy(t[:], pk[:])
                k2.append(t)

            if part == "prologue":
                # consume k2/q2 so nothing is dead
                sc = sc_pool.tile([128, TK], FP32, tag="sc")
                nc.vector.tensor_copy(sc[:], k2[0][:])
                nc.vector.tensor_copy(sc[:, :QROWS], q2[0][:])
                nc.sync.dma_start(out[0:128, :], sc[:])
                continue

            # ---- main loop ----
            no_dve = part in ("act", "act_pe")
            no_pe = part in ("act", "act_dve")
            s_static = None
            n_groups = 128 // gq  # groups per q-block
            for qb in range(QROWS // 128):
                if not no_pe:
                    psum = ps_pool.tile([128, TK], FP32, tag="psc")
                for g in range(n_groups):
                    qbase = qb * 128 + g * gq
                    if no_dve:
                        if s_static is None:
                            s_static = stage_pool.tile(
                                [128, wide], FP16 if stage_fp16 else FP32,
                                tag="stage",
                            )
                            nc.vector.memset(s_static[:], 0.125)
                        s_t = s_static
                    else:
                        s_t = stage_pool.tile(
                            [128, wide], FP16 if stage_fp16 else FP32, tag="stage"
                        )
                        for s in range(nslot):
                            q = qbase + s // NCC
                            cc = s % NCC
                            nc.vector.tensor_scalar_add(
                                s_t[:, s * TK:(s + 1) * TK],
                                k2[cc][:],
                                q2[cc][:, q:q + 1],
                            )
                    t_t = tanh_pool.tile([128, wide], FP16, tag="tanh")
                    _act_immediate(nc, t_t[:], s_t[:])
                    if no_pe:
                        # keep each tanh tile alive with a tiny probe read
                        probe = sc_pool.tile([128, 1], FP32, tag="probe")
                        nc.vector.tensor_copy(probe[:], t_t[:, :1])
                    else:
                        for s in range(nslot):
                            q = qbase + s // NCC
                            cc = s % NCC
                            qi = q - qb * 128
                            zoff = cc * 257 + 128 - qi
                            nc.tensor.matmul(
                                psum[:],
                                zw_sb[:, zoff:zoff + 128],
                                t_t[:, s * TK:(s + 1) * TK],
                                start=(g == 0 and s == 0),
                                stop=(g == n_groups - 1 and s == nslot - 1),
                            )
                sc = sc_pool.tile([128, TK], FP32, tag="sc")
                if no_pe:
                    nc.vector.tensor_copy(sc[:], t_t[:, :TK])
                else:
                    nc.vector.tensor_scalar_add(sc[:], psum[:], bb_sb[:])
                nc.sync.dma_start(out[qb * 128:(qb + 1) * 128, :], sc[:])
            s_static = None

    return nc


class SpmdRunner:
    """Persistent 8-core runner: jit/load the NEFF once, re-invoke cheaply.

    run_bass_kernel_spmd under axon rebuilds the jax.jit closure every call,
    so every invocation re-ships and re-loads the NEFF. Keeping the jitted
    executable alive makes repeated kernel() calls cost only dispatch +
    transfer + execution.
    """

    def __init__(self, nc: bass.Bass, n_cores: int, chain: int = 1):
        import jax
        from concourse import bass2jax
        from jax.experimental.shard_map import shard_map
        from jax.sharding import Mesh, PartitionSpec

        bass2jax.install_neuronx_cc_hook()
        self.jax = jax
        self.nc = nc
        self.n_cores = n_cores
        self.PartitionSpec = PartitionSpec

        partition_name = (
            nc.partition_id_tensor.name if nc.partition_id_tensor else None
        )
        in_names, out_names, out_avals, zero_outs = [], [], [], []
        for alloc in nc.m.functions[0].allocations:
            if not isinstance(alloc, mybir.MemoryLocationSet):
                continue
            name = alloc.memorylocations[0].name
            if alloc.kind == "ExternalInput":
                if name != partition_name:
                    in_names.append(name)
            elif alloc.kind == "ExternalOutput":
                out_names.append(name)
                shape = tuple(alloc.tensor_shape)
                dtype = mybir.dt.np(alloc.dtype)
                out_avals.append(jax.core.ShapedArray(shape, dtype))
                zero_outs.append(np.zeros(shape, dtype))
        self.in_names = list(in_names)
        self.out_names = out_names
        self.out_avals = out_avals
        self.zero_outs = zero_outs
        n_params = len(in_names)
        n_outs = len(out_avals)
        all_in_names = list(in_names) + list(out_names)
        if partition_name is not None:
            all_in_names.append(partition_name)

        def _exec(operands):
            if partition_name is not None:
                operands = operands + [bass2jax.partition_id_tensor()]
            return bass2jax._bass_exec_p.bind(
                *operands,
                out_avals=tuple(out_avals),
                in_names=tuple(all_in_names),
                out_names=tuple(out_names),
                lowering_input_output_aliases=(),
                sim_require_finite=True,
                sim_require_nnan=True,
                nc=nc,
            )

        def _body(*args):
            ins = list(args[:n_params])
            outs = list(args[n_params:])
            # Chain NEFF executions inside one dispatch: each iteration's
            # outputs seed the next call's output operands, creating a data
            # dependence so XLA cannot CSE or reorder the calls. The kernel
            # overwrites every output element, so results are unchanged.
            for _ in range(chain):
                outs = list(_exec(ins + outs))
            return tuple(outs)

        devices = jax.devices()[:n_cores]
        assert len(devices) == n_cores
        self.mesh = Mesh(np.asarray(devices), ("core",))
        in_specs = (PartitionSpec("core"),) * (n_params + n_outs)
        out_specs = (PartitionSpec("core"),) * n_outs
        self.sharded = jax.jit(
            shard_map(
                _body,
                mesh=self.mesh,
                in_specs=in_specs,
                out_specs=out_specs,
                check_rep=False,
            ),
            keep_unused=True,
        )
        self._zeros_dev = None

    def set_inputs(self, in_maps):
        jax = self.jax
        concat_in = [
            np.concatenate(
                [np.asarray(in_maps[c][name]) for c in range(self.n_cores)], axis=0
            )
            for name in self.in_names
        ]
        sharding = jax.sharding.NamedSharding(self.mesh, self.PartitionSpec("core"))
        dev_in = [jax.device_put(a, sharding) for a in concat_in]
        if self._zeros_dev is None:
            concat_zeros = [
                np.zeros((self.n_cores * z.shape[0], *z.shape[1:]), z.dtype)
                for z in self.zero_outs
            ]
            self._zeros_dev = [jax.device_put(a, sharding) for a in concat_zeros]
        self._dev_args = dev_in + self._zeros_dev
        jax.block_until_ready(self._dev_args)

    def run(self):
        out_arrs = self.sharded(*self._dev_args)
        self.jax.block_until_ready(out_arrs)
        return out_arrs

    def results(self, out_arrs):
        res = []
        for c in range(self.n_cores):
            res.append(
                {
                    name: np.asarray(out_arrs[i]).reshape(
                        self.n_cores, *self.out_avals[i].shape
                    )[c]
                    for i, name in enumerate(self.out_names)
                }
            )
        return res


_RUNNER_CACHE = None


def _get_runner():
    global _RUNNER_CACHE
    if _RUNNER_CACHE is None:
        _RUNNER_CACHE = SpmdRunner(build_program(), N_CORES)
    return _RUNNER_CACHE


def make_in_maps(query, key, Wq, Wk, w_attn, b_attn, in16: bool = True):
    in_np = np.float16 if in16 else np.float32
    w16 = np.asarray(w_attn, dtype=np.float16)
    zw = np.zeros((128, NCC * 257), dtype=np.float16)
    for cc in range(NCC):
        zw[:, cc * 257 + 128] = w16[cc * 128:(cc + 1) * 128]
    bbv = np.full((128, 1), np.float32(b_attn), dtype=np.float32)
    wq = np.ascontiguousarray(np.asarray(Wq, dtype=in_np))
    wk = np.ascontiguousarray(np.asarray(Wk, dtype=in_np))

    in_maps = []
    for i in range(N_CORES):
        b = i // 2
        h = i % 2
        qs = np.ascontiguousarray(
            np.asarray(query[b, h * QROWS:(h + 1) * QROWS, :], dtype=in_np).T
        )
        ks = np.ascontiguousarray(np.asarray(key[b], dtype=in_np).T)
        in_maps.append(
            {"qT": qs, "kT": ks, "wq": wq, "wk": wk, "zw": zw, "bb": bbv}
        )
    return in_maps


def kernel(query, key, Wq, Wk, w_attn, b_attn):
    r = _get_runner()
    in_maps = make_in_maps(query, key, Wq, Wk, w_attn, b_attn)
    r.set_inputs(in_maps)
    res = r.results(r.run())
    scores = np.empty((B, TQ, TK), dtype=np.float32)
    for i in range(N_CORES):
        b = i // 2
        h = i % 2
        scores[b, h * QROWS:(h + 1) * QROWS, :] = res[i]["out"]
    return scores
